# revision 8
# baseline (speedup 1.0000x reference)
"""KAN layer Trainium2 kernel.

Math: out[b,o] = sum_{i,g} exp(-|tanh(x[b,i]) - grid[g]| * s[o,i]) * w[o,i,g]

For t = tanh(x) in grid interval v (grid[v] <= t < grid[v+1]):
    f_{o,i}(t) = P_v * e^{-s t} + S_{v+1} * e^{s t}
with P_v = sum_{j<=v} w_j e^{s g_j}, S_{v+1} = sum_{j>v} w_j e^{-s g_j}.
Each piece is expanded in a degree-(NCHEB-1) Chebyshev basis of the
interval-local coordinate d = 7t + 6 - 2v, giving

    out[b,o] = sum_{i,v,c} mask_v(t[b,i]) * T_c(d[b,i]) * D[(v,c,i), o]

i.e. one (B x NV*NCHEB*I) @ (NV*NCHEB*I x O) matmul, 8-way data-parallel
over the batch. D is precomputed on the host (weight-only preprocessing).

vs the fp32 NCHEB=7 version: both matmul operands are bf16 (fp32 matmuls
issue as 2 HI/LO instructions with slow weight loads; bf16 gets FWL),
NCHEB=4 (quantization to bf16 floors the error at ~1.7e-3 anyway), the
interval index k and local coordinate d come from a float `mod` (4 ops
instead of 13), the 28 mask*T_c products per i-half are single fused
scalar_tensor_tensor ops split across VectorE/GpSimd, and dummy warm-up
matmuls ramp the PE clock gate during the basis computation.
"""

import numpy as np
import ml_dtypes

B, I, O, G = 1024, 256, 256, 8
NV = G - 1            # 7 intervals
NCHEB = 4             # degree-3 Chebyshev per interval
N_CORES = 8
BSH = B // N_CORES    # 128 batch rows per core
IH = I // 128         # 2 partition halves of the i dimension
NWARM = 4             # dummy matmuls to start the PE clock-gate ramp

_CACHE = {}


def _precompute_dmat(spline_weight, spline_scaler, grid):
    """D[(v,c,ih,i128), o] coefficients, bf16, shape (NV, IH, 128, NCHEB*O)."""
    w = spline_weight.astype(np.float64)          # (O, I, G)
    s = spline_scaler.astype(np.float64)          # (O, I)
    g = grid.astype(np.float64)                   # (G,)

    Eg = np.exp(g[None, None, :] * s[:, :, None])             # (O,I,G)
    P = np.cumsum(w * Eg, axis=2)                              # prefix_j<=v
    S = np.cumsum((w / Eg)[:, :, ::-1], axis=2)[:, :, ::-1]    # suffix_j>=v

    h = 1.0 / NV
    centers = -1.0 + (2 * np.arange(NV) + 1) * h

    # Chebyshev coefs of e^{-s h d}, d in [-1,1], via node projection
    M = 32
    nodes = np.cos(np.pi * (np.arange(M) + 0.5) / M)
    Tn = np.cos(np.outer(np.arange(NCHEB), np.arccos(nodes)))  # (NCHEB, M)
    proj = Tn.T * (2.0 / M)
    proj[:, 0] *= 0.5
    sf = s.reshape(-1)                                          # (O*I,)
    Fm = np.exp(-np.outer(sf * h, nodes))                       # (O*I, M)
    Am = Fm @ proj                                              # coefs of e^{-s h d}
    Ap = (1.0 / Fm) @ proj                                      # coefs of e^{+s h d}

    Pf = P.reshape(O * I, G)
    Sf = S.reshape(O * I, G)
    D = np.empty((NV, NCHEB, O * I))
    for v in range(NV):
        em = np.exp(-sf * centers[v])
        pc = Pf[:, v] * em
        sc = Sf[:, v + 1] / em
        D[v] = (pc[:, None] * Am + sc[:, None] * Ap).T          # (NCHEB, O*I)
    # (NV, NCHEB, O, I) -> (NV, IH, 128, NCHEB*O): one big DMA per (v, ihalf)
    Dm = D.reshape(NV, NCHEB, O, IH, 128).transpose(0, 3, 4, 1, 2)
    Dm = Dm.reshape(NV, IH, 128, NCHEB * O)
    return np.ascontiguousarray(Dm).astype(ml_dtypes.bfloat16)


def _build_module():
    import concourse.bacc as bacc
    import concourse.bass as bass
    import concourse.mybir as mybir
    import concourse.tile as tile

    f32 = mybir.dt.float32
    bf16 = mybir.dt.bfloat16
    AF = mybir.ActivationFunctionType
    ALU = mybir.AluOpType

    nc = bacc.Bacc("TRN2", target_bir_lowering=False, debug=False,
                   num_devices=N_CORES)

    xT = nc.dram_tensor("xt", [IH, 128, BSH], f32, kind="ExternalInput")
    dmat = nc.dram_tensor("dmat", [NV, IH, 128, NCHEB * O], bf16,
                          kind="ExternalInput")
    out_d = nc.dram_tensor("out", [BSH, O], f32, kind="ExternalOutput")

    with tile.TileContext(nc) as tc:
        with (
            tc.tile_pool(name="keep", bufs=1) as keep,
            tc.tile_pool(name="dpool", bufs=NV * IH) as dpool,
            tc.tile_pool(name="prod", bufs=16) as prod,
            tc.tile_pool(name="obuf", bufs=1) as obuf,
            tc.tile_pool(name="psum", bufs=1, space=bass.MemorySpace.PSUM) as ppool,
        ):
            # x tiles first on the DMA queue, then all D chunks (no deps).
            xsb = [None] * IH
            for hh in range(IH):
                xsb[hh] = keep.tile([128, BSH], f32, tag=f"x{hh}", name=f"x{hh}")
                nc.sync.dma_start(xsb[hh][:], xT[hh])
            dsb = [[None] * IH for _ in range(NV)]
            for v in range(NV):
                for hh in range(IH):
                    dsb[v][hh] = dpool.tile([128, NCHEB * O], bf16, tag="d",
                                            name=f"dsb{v}_{hh}")
                    nc.sync.dma_start(dsb[v][hh][:], dmat[v, hh])

            # Dummy matmuls on a zeroed tile keep the PE active while the
            # basis is being computed, so the 2.4 GHz clock gate opens
            # before the real matmul stream starts.
            wz = keep.tile([128, 512], bf16, tag="warm", name="warm")
            nc.vector.memset(wz[:], 0.0)
            wps = ppool.tile([128, 512], f32, tag="wps", name="wps")
            for _ in range(NWARM):
                nc.tensor.matmul(wps[:], wz[:, :128], wz[:],
                                 start=True, stop=True)

            # Basis per i-half: t = tanh(x); k = round(3.5t + 3) via the
            # +M/-M fp32 trick with M = 1.5*2^23 (spacing exactly 1.0 there;
            # round(3.5t+3) = floor(3.5t+3.5) = interval index 0..6; RTE
            # ties at grid points are harmless). d = (7t+6) - 2k in [-1,1];
            # T2 = 2d^2-1; T3 = d(2 T2 - 1). All immediates are bf16-exact
            # (the TS immediate path quantizes: 12582911.5 behaves as M).
            kb = [None] * IH
            cheb = [[None] * NCHEB for _ in range(IH)]
            MAGIC = 12582912.0  # 1.5 * 2^23
            for hh in range(IH):
                t = keep.tile([128, BSH], f32, tag=f"t{hh}")
                nc.scalar.activation(t[:], xsb[hh][:], AF.Tanh)
                ua = keep.tile([128, BSH], f32, tag=f"ua{hh}")
                nc.vector.tensor_scalar(ua[:], t[:], 3.5, 3.0, ALU.mult, ALU.add)
                r1 = keep.tile([128, BSH], f32, tag=f"r1{hh}")
                nc.vector.tensor_scalar(r1[:], ua[:], MAGIC, None, ALU.add)
                kf = keep.tile([128, BSH], f32, tag=f"kf{hh}")
                nc.vector.tensor_scalar(kf[:], r1[:], MAGIC, None, ALU.subtract)
                kk = keep.tile([128, BSH], bf16, tag=f"k{hh}")
                nc.vector.tensor_scalar(kk[:], r1[:], MAGIC, None, ALU.subtract)
                kb[hh] = kk
                u7 = keep.tile([128, BSH], f32, tag=f"u7{hh}")
                nc.vector.tensor_scalar(u7[:], t[:], 7.0, 6.0, ALU.mult, ALU.add)
                db = keep.tile([128, BSH], bf16, tag=f"d{hh}")
                nc.vector.scalar_tensor_tensor(db[:], kf[:], -2.0, u7[:],
                                               ALU.mult, ALU.add)
                d2 = keep.tile([128, BSH], bf16, tag=f"d2{hh}")
                nc.vector.tensor_tensor(d2[:], db[:], db[:], ALU.mult)
                t2 = keep.tile([128, BSH], bf16, tag=f"T2{hh}")
                nc.vector.tensor_scalar(t2[:], d2[:], 2.0, -1.0, ALU.mult, ALU.add)
                u3 = keep.tile([128, BSH], bf16, tag=f"u3{hh}")
                nc.vector.tensor_scalar(u3[:], t2[:], 2.0, -1.0, ALU.mult, ALU.add)
                t3 = keep.tile([128, BSH], bf16, tag=f"T3{hh}")
                nc.vector.tensor_tensor(t3[:], db[:], u3[:], ALU.mult)
                cheb[hh] = [None, db, t2, t3]

            # Products + matmul stream. lhs tile for (v,c,hh) is
            # (k==v) * T_c as one fused op; PE accumulates into PSUM.
            acc = ppool.tile([BSH, O], f32, tag="acc", name="acc")
            n_chunks = NV * NCHEB * IH
            idx = 0
            for v in range(NV):
                for c in range(NCHEB):
                    for hh in range(IH):
                        pt = prod.tile([128, BSH], bf16, tag="p", name=f"p{idx}")
                        eng = nc.vector
                        if c == 0:
                            eng.tensor_scalar(pt[:], kb[hh][:], float(v), None,
                                              ALU.is_equal)
                        else:
                            eng.scalar_tensor_tensor(pt[:], kb[hh][:], float(v),
                                                     cheb[hh][c][:],
                                                     ALU.is_equal, ALU.mult)
                        nc.tensor.matmul(acc[:], pt[:],
                                         dsb[v][hh][:, c * O:(c + 1) * O],
                                         start=(idx == 0),
                                         stop=(idx == n_chunks - 1))
                        idx += 1

            osb = obuf.tile([BSH, O], f32, tag="o", name="o")
            nc.scalar.copy(osb[:], acc[:])
            nc.sync.dma_start(out_d[:], osb[:])

    nc.compile()
    return nc


def kernel(x, spline_weight, spline_scaler, grid):
    from concourse import bass_utils

    x = np.asarray(x, dtype=np.float32)
    Dm = _precompute_dmat(np.asarray(spline_weight), np.asarray(spline_scaler),
                          np.asarray(grid))

    if "nc" not in _CACHE:
        _CACHE["nc"] = _build_module()
    nc = _CACHE["nc"]

    in_maps = []
    for cid in range(N_CORES):
        xs = x[cid * BSH:(cid + 1) * BSH]                  # (BSH, I)
        xT = np.ascontiguousarray(xs.T.reshape(IH, 128, BSH), dtype=np.float32)
        in_maps.append({"xt": xT, "dmat": Dm})

    import os
    trace = bool(int(os.environ.get("KAN_TRACE", "0")))
    kw = {}
    if trace:
        tdir = os.environ.get("KAN_TRACE_DIR") or None
        kw = dict(trace=True, tmpdir=tdir)
    res = bass_utils.run_bass_kernel_spmd(nc, in_maps,
                                          core_ids=list(range(N_CORES)), **kw)
    _CACHE["last_result"] = res
    out = np.concatenate([res.results[cid]["out"] for cid in range(N_CORES)], axis=0)
    return out.astype(np.float32)
